# revision 14
# baseline (speedup 1.0000x reference)
"""GATNet (2-layer GAT, 8 heads->1 head) on 8 trn2 NeuronCores.

Strategy: dst-node sharding (pull mode). Each core owns N/8 dst nodes and
all edges into them (sorted by dst, grouped into 128-dst blocks, 128-edge
chunks). Dense per-node work (x@W1, attention dots) is replicated on all
cores; per-edge work uses dma_gather (SWDGE) of packed per-node records,
segment-softmax via indicator matmuls accumulating in PSUM, and one
AllGather at the layer boundary for the layer-2 node records.
"""
import inspect
import textwrap

import ml_dtypes
import numpy as np

import concourse.bass as bass
import concourse.bacc as bacc
import concourse.mybir as mybir
import concourse.tile as tile
from concourse import bass_utils
from concourse.bass import AP

# ---- relax dma_gather's elem_size%256 assert (stride stays %256 = ISA field;
# 288B/36B payloads verified on HW) ----
_src = textwrap.dedent(inspect.getsource(bass.BassGpSimd.dma_gather))
_old = """    assert (
        elem_size_bytes > 0 and elem_size_bytes % 256 == 0
    )  # transpose restriction"""
_new = "    assert elem_size_bytes > 0 and (elem_size_bytes % 256 == 0 or (not transpose and elem_size_bytes % 4 == 0))"
assert _old in _src
_ns = {}
exec(compile(_src.replace(_old, _new), "<patched_dma_gather>", "exec"), bass.__dict__, _ns)
bass.BassGpSimd.dma_gather = _ns["dma_gather"]

F32 = mybir.dt.float32
BF16 = mybir.dt.bfloat16
I16 = mybir.dt.int16
I8 = mybir.dt.int8
AF = mybir.ActivationFunctionType
OP = mybir.AluOpType

NEG = 0.2
NCORES = 8
LAST_EXEC_NS = None
SHARD = 32768  # int16 index limit


def _ceil(a, b):
    return -(-a // b)


def _prep_core(src, dst, lo, hi, nblk):
    """Per-core host prep: edges with dst in [lo,hi), dst-sorted on entry.
    Returns per-block chunk lists: (idxA, idxB, segcol, nA, nB) unpadded."""
    blocks = []
    for b in range(nblk):
        dlo, dhi = lo + b * 128, min(lo + (b + 1) * 128, hi)
        i0, i1 = np.searchsorted(dst, [dlo, dhi])
        s, d = src[i0:i1], dst[i0:i1]
        a_mask = s < SHARD
        sa, da = s[a_mask], d[a_mask]
        sb, db = s[~a_mask] - SHARD, d[~a_mask]
        blocks.append(((sa, da - dlo + 0), (sb, db - dlo), dlo - lo))
    return blocks


def _pad_chunks(s, seg, nchunks):
    """Pad edge list to nchunks*128; idx=0, segid=-1 for pad lanes."""
    tot = nchunks * 128
    idx = np.zeros(tot, np.int16)
    sg = np.full(tot, -1.0, np.float32)
    idx[: len(s)] = s.astype(np.int16)
    sg[: len(s)] = seg.astype(np.float32)
    return idx, sg


def _wrap_idx(idx):
    """[n*128] int16 -> [128, n*8] wrapped (16-partition) + replicated 8x."""
    n = len(idx) // 128
    w = idx.reshape(n * 8, 16).T  # [16, n*8]: pos (p, s) = idx[s*16+p]
    return np.tile(w, (8, 1)).copy()  # [128, n*8]


def build_host_data(inputs, n, ncores):
    x = np.asarray(inputs["x"], np.float32)
    ei = np.asarray(inputs["edge_index"])
    W1 = np.asarray(inputs["W1"], np.float32)
    a_src1 = np.asarray(inputs["a_src1"], np.float32)
    a_dst1 = np.asarray(inputs["a_dst1"], np.float32)
    b1 = np.asarray(inputs["b1"], np.float32)
    W2 = np.asarray(inputs["W2"], np.float32)
    a_src2 = np.asarray(inputs["a_src2"], np.float32)
    a_dst2 = np.asarray(inputs["a_dst2"], np.float32)
    b2 = np.asarray(inputs["b2"], np.float32)
    H1, HID = a_src1.shape
    OUTD = a_src2.shape[1]

    loop = np.arange(n, dtype=np.int64)
    src = np.concatenate([ei[0].astype(np.int64), loop])
    dst = np.concatenate([ei[1].astype(np.int64), loop])
    order = np.argsort(dst, kind="stable")
    src, dst = src[order], dst[order]

    nloc = n // ncores
    nblk = _ceil(nloc, 128)
    cores = [_prep_core(src, dst, c * nloc, (c + 1) * nloc, nblk) for c in range(ncores)]

    # uniform (ncA, ncB) per block across cores (SPMD: one program)
    ncA = [max(_ceil(max(len(cores[c][b][0][0]), 1), 128) for c in range(ncores)) for b in range(nblk)]
    ncB = [max(_ceil(max(len(cores[c][b][1][0]), 1), 128) for c in range(ncores)) for b in range(nblk)]

    # flat streams per core
    per_core = []
    for c in range(ncores):
        idxA_l, idxB_l, segc_l, segb_l = [], [], [], []
        for b in range(nblk):
            (sa, ga), (sb, gb), _ = cores[c][b]
            ia, sga = _pad_chunks(sa, ga, ncA[b])
            ib, sgb = _pad_chunks(sb, gb, ncB[b])
            idxA_l.append(_wrap_idx(ia))
            idxB_l.append(_wrap_idx(ib))
            seg = np.concatenate([sga, sgb])  # [(ncA+ncB)*128]
            nc_t = ncA[b] + ncB[b]
            segc_l.append(seg.reshape(nc_t, 128).T.copy())  # [128 lanes, nc_t]
            segb_l.append(np.broadcast_to(seg.astype(ml_dtypes.bfloat16).reshape(1, nc_t * 128), (128, nc_t * 128)))
        per_core.append(dict(
            idxA=np.concatenate(idxA_l, axis=1),          # [128, sum(ncA)*8] i16
            idxB=np.concatenate(idxB_l, axis=1),          # [128, sum(ncB)*8] i16
            segcol=np.concatenate(segc_l, axis=1).astype(np.float32),  # [128, sum(nc)]
            segbcast=np.concatenate(segb_l, axis=1).astype(ml_dtypes.bfloat16),
        ))

    # fused weights
    # s_src1[n,h] = sum_c h[n,h,c]*a_src1[h,c] = x[n] @ (W1[:, h*HID:(h+1)*HID] @ a_src1[h])
    wsrc1 = np.stack([W1[:, h * HID:(h + 1) * HID] @ a_src1[h] for h in range(H1)], axis=1)  # [128, H1]
    wdst1 = np.stack([W1[:, h * HID:(h + 1) * HID] @ a_dst1[h] for h in range(H1)], axis=1)
    W1cat = np.concatenate([W1, wsrc1, wdst1], axis=1)  # [128, 144]
    W2cat = np.concatenate([W2, W2 @ a_src2[0][:, None], W2 @ a_dst2[0][:, None]], axis=1)  # [128, 18]

    consts = dict(
        xT=np.ascontiguousarray(x.T).astype(ml_dtypes.bfloat16),  # [128, n]
        W1cat=W1cat.astype(ml_dtypes.bfloat16),
        W2cat=W2cat.astype(ml_dtypes.bfloat16),
        iota_row=np.broadcast_to(np.arange(128, dtype=np.float32), (128, 128)).copy(),
        iota_col=np.arange(128, dtype=np.float32).reshape(128, 1).copy(),
        ident=np.eye(128, dtype=np.float32).astype(ml_dtypes.bfloat16),
        b1_bcast=np.broadcast_to(b1, (128, H1 * HID)).copy().astype(np.float32),
        b2_bcast=np.broadcast_to(b2, (128, OUTD)).copy().astype(np.float32),
    )
    meta = dict(n=n, nloc=nloc, nblk=nblk, ncA=ncA, ncB=ncB, H1=H1, HID=HID, OUTD=OUTD,
                totA=sum(ncA), totB=sum(ncB), tot=sum(ncA) + sum(ncB))
    return consts, per_core, meta


def bcast_inner(ap, count):
    """Append a step-0 dim of size count to an AP (free-dim broadcast)."""
    return AP(ap.tensor, ap.offset, list(ap.ap) + [[0, count]])


def build_kernel(meta):
    n, nloc, nblk = meta["n"], meta["nloc"], meta["nblk"]
    H1, HID, OUTD = meta["H1"], meta["HID"], meta["OUTD"]
    F1 = H1 * HID  # 128
    ncA, ncB = meta["ncA"], meta["ncB"]
    NCMAX = max(ncA[b] + ncB[b] for b in range(nblk))
    NTILE = _ceil(n, 128)

    nc = bacc.Bacc(None, target_bir_lowering=False, num_devices=NCORES, num_swdge_queues=4)
    # inputs
    xT = nc.dram_tensor("xT", [128, n], BF16, kind="ExternalInput")
    W1c = nc.dram_tensor("W1cat", [128, F1 + 2 * H1], BF16, kind="ExternalInput")
    W2c = nc.dram_tensor("W2cat", [F1, OUTD + 2], BF16, kind="ExternalInput")
    iota_row_d = nc.dram_tensor("iota_row", [128, 128], F32, kind="ExternalInput")
    iota_col_d = nc.dram_tensor("iota_col", [128, 1], F32, kind="ExternalInput")
    ident_d = nc.dram_tensor("ident", [128, 128], BF16, kind="ExternalInput")
    b1_d = nc.dram_tensor("b1_bcast", [128, F1], F32, kind="ExternalInput")
    b2_d = nc.dram_tensor("b2_bcast", [128, OUTD], F32, kind="ExternalInput")
    xTl = nc.dram_tensor("xTl", [128, nloc], BF16, kind="ExternalInput")
    idxA_d = nc.dram_tensor("idxA", [128, meta["totA"] * 8], I16, kind="ExternalInput")
    idxB_d = nc.dram_tensor("idxB", [128, meta["totB"] * 8], I16, kind="ExternalInput")
    segcol_d = nc.dram_tensor("segcol", [128, meta["tot"]], F32, kind="ExternalInput")
    segb_d = nc.dram_tensor("segbcast", [128, meta["tot"] * 128], BF16, kind="ExternalInput")
    out_d = nc.dram_tensor("out", [nloc, OUTD], F32, kind="ExternalOutput")
    dbg_d = nc.dram_tensor("dbg_rec2", [nloc, 24], BF16, kind="ExternalOutput")
    dbg2_d = nc.dram_tensor("dbg_den", [nloc, 8], F32, kind="ExternalOutput")
    # internal DRAM
    T1 = nc.dram_tensor("T1", [NTILE * 128, 256], BF16, kind="Internal")
    rec2l = nc.dram_tensor("rec2l", [nloc, 128], BF16, kind="Internal")
    SD2 = nc.dram_tensor("SD2", [nblk * 128, 1], BF16, kind="Internal")
    T2 = nc.dram_tensor("T2", [n, 128], BF16, kind="Internal", addr_space="Shared")

    REC1 = F1 + 2 * H1  # 144 bf16 slots (h bf16 + s_src f32x8 as 16 slots)
    REC2 = OUTD + 4     # 20 bf16 slots (h2 bf16 x16 + s2src f32 as 2)... see below
    qn = [0]

    def nxq():
        qn[0] = (qn[0] + 1) % 4
        return qn[0]

    with tile.TileContext(nc) as tc:
        with (
            tc.tile_pool(name="const", bufs=1) as cp,
            tc.tile_pool(name="dense", bufs=3) as dp,
            tc.tile_pool(name="dpsum", bufs=2, space="PSUM") as dpp,
            tc.tile_pool(name="edge", bufs=2) as ep,
            tc.tile_pool(name="epsum", bufs=2, space="PSUM") as epp,
            tc.tile_pool(name="epsum2", bufs=1, space="PSUM") as epp2,
            tc.tile_pool(name="blk", bufs=2) as bp,
        ):
            w1t = cp.tile([128, F1 + 2 * H1], BF16)
            nc.sync.dma_start(w1t[:], W1c[:, :])
            w2t = cp.tile([F1, OUTD + 2], BF16)
            nc.sync.dma_start(w2t[:], W2c[:, :])
            irow = cp.tile([128, 128], F32)
            nc.sync.dma_start(irow[:], iota_row_d[:, :])
            icol = cp.tile([128, 1], F32)
            nc.sync.dma_start(icol[:], iota_col_d[:, :])
            idt = cp.tile([128, 128], BF16)
            nc.sync.dma_start(idt[:], ident_d[:, :])
            b1t = cp.tile([128, F1], F32)
            nc.sync.dma_start(b1t[:], b1_d[:, :])
            b2t = cp.tile([128, OUTD], F32)
            nc.sync.dma_start(b2t[:], b2_d[:, :])

            # ---------- dense phase: T1 records + SD1, all nodes ----------
            for t in range(NTILE):
                m = min(128, n - t * 128)
                xt = dp.tile([128, 128], BF16, tag="xt")
                nc.sync.dma_start(xt[:, :m], xT[:, t * 128:t * 128 + m])
                ps = dpp.tile([128, F1 + 2 * H1], F32, tag="dps")
                nc.tensor.matmul(ps[:m, :], xt[:, :m], w1t[:], start=True, stop=True)
                stg = dp.tile([128, REC1], BF16, tag="stg")
                nc.scalar.activation(stg[:m, :F1], ps[:m, :F1], AF.Copy)
                stg32 = stg[:].bitcast(F32)
                nc.vector.tensor_copy(stg32[:m, F1 // 2:F1 // 2 + H1], ps[:m, F1:F1 + H1])
                nc.sync.dma_start(T1[t * 128:t * 128 + m, :REC1], stg[:m, :])

            # ---------- edge phase helper ----------
            def edge_layer(layer, T, rec_w, nheads, fdim, SD, finalize):
                """rec_w: bf16 slots gathered per edge; h part = fdim*? ..."""
                offA = offB = offC = 0
                for b in range(nblk):
                    na, nb_ = ncA[b], ncB[b]
                    nct = na + nb_
                    dvalid = min(128, nloc - b * 128)
                    # block loads
                    sdb = bp.tile([128, nheads], BF16, tag=f"sdb{layer}")
                    if dvalid < 128:
                        nc.vector.memset(sdb[:, :], 0.0)
                    if layer == 1:
                        xb = bp.tile([128, 128], BF16, tag="xb")
                        nc.sync.dma_start(xb[:, :dvalid], xTl[:, b * 128:b * 128 + dvalid])
                        psdb = epp2.tile([128, OUTD + 2], F32, tag="ps2")
                        nc.tensor.matmul(psdb[:dvalid, :nheads], xb[:, :dvalid],
                                         w1t[:, F1 + H1:F1 + 2 * H1], start=True, stop=True)
                        nc.scalar.activation(sdb[:dvalid, :], psdb[:dvalid, :nheads], AF.Copy)
                    else:
                        nc.sync.dma_start(sdb[:dvalid, :], SD[b * 128:b * 128 + dvalid, :])
                    it = ep.tile([128, NCMAX * 8], I16, tag=f"it{layer}")
                    nc.sync.dma_start(it[:, :na * 8], idxA_d[:, offA * 8:(offA + na) * 8])
                    nc.sync.dma_start(it[:, na * 8:nct * 8], idxB_d[:, offB * 8:(offB + nb_) * 8])
                    scol = ep.tile([128, NCMAX], F32, tag=f"sc{layer}")
                    nc.sync.dma_start(scol[:, :nct], segcol_d[:, offC:offC + nct])
                    sbc = ep.tile([128, NCMAX * 128], BF16, tag=f"sb{layer}")
                    nc.sync.dma_start(sbc[:, :nct * 128], segb_d[:, offC * 128:(offC + nct) * 128])
                    # gathers (A then B shard)
                    g = ep.tile([128, NCMAX * rec_w], BF16, tag=f"g{layer}")
                    TA = T[:SHARD, :rec_w] if T.shape[0] > SHARD else T[:, :rec_w]
                    nc.gpsimd.dma_gather(
                        g[:, :na * rec_w].rearrange("p (t e) -> p t e", e=rec_w),
                        TA, it[:, :na * 8],
                        num_idxs=na * 128, num_idxs_reg=na * 128, elem_size=rec_w,
                        elem_step=T.shape[1], single_packet=False, queue_num=nxq())
                    if n > SHARD:
                        nc.gpsimd.dma_gather(
                            g[:, na * rec_w:nct * rec_w].rearrange("p (t e) -> p t e", e=rec_w),
                            T[SHARD:, :rec_w], it[:, na * 8:nct * 8],
                            num_idxs=nb_ * 128, num_idxs_reg=nb_ * 128, elem_size=rec_w,
                            elem_step=T.shape[1], single_packet=False, queue_num=nxq())
                    else:
                        nc.gpsimd.dma_gather(
                            g[:, na * rec_w:nct * rec_w].rearrange("p (t e) -> p t e", e=rec_w),
                            T[:, :rec_w], it[:, na * 8:nct * 8],
                            num_idxs=nb_ * 128, num_idxs_reg=nb_ * 128, elem_size=rec_w,
                            elem_step=T.shape[1], single_packet=False, queue_num=nxq())
                    # indicators
                    indT = ep.tile([128, NCMAX * 128], BF16, tag=f"iT{layer}")
                    nc.vector.tensor_scalar(indT[:, :nct * 128], sbc[:, :nct * 128], icol[:, :1], None, op0=OP.is_equal)
                    ind = ep.tile([128, NCMAX * 128], BF16, tag=f"ind{layer}")
                    nc.vector.tensor_tensor(
                        ind[:].rearrange("p (c e) -> p c e", e=128)[:, :nct, :],
                        bcast_inner(scol[:, :nct], 128),
                        AP(irow[:].tensor, irow[:].offset, [irow[:].ap[0], [0, nct], irow[:].ap[1]]),
                        op=OP.is_equal)
                    # s_dst expansion: per chunk matmul
                    pssd = epp.tile([128, NCMAX * 8], F32, tag="psd")
                    for ci in range(nct):
                        nc.tensor.matmul(pssd[:, ci * nheads:(ci + 1) * nheads],
                                         indT[:, ci * 128:(ci + 1) * 128], sdb[:, :],
                                         start=True, stop=True)
                    sD = ep.tile([128, NCMAX * nheads], F32, tag=f"sD{layer}")
                    nc.vector.tensor_copy(sD[:, :nct * nheads], pssd[:, :nct * nheads])
                    # e = lrelu(sS + sD); ee = exp(e)  (batched per block)
                    g32 = g[:].bitcast(F32)
                    sSv = g32[:, :nct * (rec_w // 2)].rearrange(
                        "p (c w) -> p c w", w=rec_w // 2)[:, :, fdim // 2:fdim // 2 + nheads]
                    et = ep.tile([128, NCMAX * nheads], F32, tag=f"et{layer}")
                    nc.vector.tensor_tensor(
                        et[:, :nct * nheads].rearrange("p (c h) -> p c h", h=nheads),
                        sSv, sD[:, :nct * nheads].rearrange("p (c h) -> p c h", h=nheads), op=OP.add)
                    nc.vector.scalar_tensor_tensor(et[:, :nct * nheads], et[:, :nct * nheads], NEG,
                                                   et[:, :nct * nheads], op0=OP.mult, op1=OP.max)
                    vw = fdim + nheads
                    chan = fdim // nheads
                    EE = ep.tile([128, NCMAX * nheads], BF16, tag=f"EE{layer}")
                    nc.scalar.activation(EE[:, :nct * nheads], et[:, :nct * nheads], AF.Exp)
                    V = ep.tile([128, NCMAX * fdim], BF16, tag=f"V{layer}")
                    nc.vector.tensor_tensor(
                        V[:, :nct * fdim].rearrange("p (c h x) -> p c h x", h=nheads, x=chan),
                        g[:, :nct * rec_w].rearrange("p (c w) -> p c w", w=rec_w)[:, :, :fdim].rearrange("p c (h x) -> p c h x", x=chan),
                        bcast_inner(EE[:, :nct * nheads].rearrange("p (c h) -> p c h", h=nheads), chan),
                        op=OP.mult)
                    # aggregation
                    psb = epp.tile([128, 136], F32, tag="pb")
                    for ci in range(nct):
                        nc.tensor.matmul(psb[:, :fdim], ind[:, ci * 128:(ci + 1) * 128],
                                         V[:, ci * fdim:(ci + 1) * fdim],
                                         start=(ci == 0), stop=(ci == nct - 1))
                    for ci in range(nct):
                        nc.tensor.matmul(psb[:, fdim:vw], ind[:, ci * 128:(ci + 1) * 128],
                                         EE[:, ci * nheads:(ci + 1) * nheads],
                                         start=(ci == 0), stop=(ci == nct - 1))
                    finalize(b, psb, dvalid)
                    offA += na; offB += nb_; offC += nct

            # ---------- L1 finalize: normalize + elu -> rec2 ----------
            def fin1(b, psb, dvalid):
                dbt = bp.tile([128, H1], F32, tag="dbt")
                nc.vector.tensor_copy(dbt[:], psb[:, F1:F1 + H1])
                nc.sync.dma_start(dbg2_d[b * 128:b * 128 + dvalid, :], dbt[:dvalid, :])
                r = bp.tile([128, H1], F32, tag="r1")
                nc.vector.reciprocal(r[:, :], psb[:, F1:F1 + H1])
                o1 = bp.tile([128, F1], F32, tag="o1")
                nc.vector.tensor_tensor(
                    o1[:].rearrange("p (h x) -> p h x", x=HID),
                    psb[:, :F1].rearrange("p (h x) -> p h x", x=HID),
                    bcast_inner(r[:, :], HID), op=OP.mult)
                nc.vector.tensor_tensor(o1[:], o1[:], b1t[:], op=OP.add)
                # elu = max(x,0) + exp(min(x,0)) - 1
                neg = bp.tile([128, F1], F32, tag="neg")
                nc.vector.tensor_scalar(neg[:], o1[:], 0.0, None, op0=OP.min)
                ex = bp.tile([128, F1], F32, tag="ex")
                nc.scalar.activation(ex[:], neg[:], AF.Exp)
                nc.vector.tensor_scalar(o1[:], o1[:], 0.0, None, op0=OP.max)
                elu = bp.tile([128, F1], BF16, tag="elu")
                nc.vector.scalar_tensor_tensor(elu[:], o1[:], -1.0, ex[:], op0=OP.add, op1=OP.add)
                # transpose -> rec2 = [h2 bf16 x OUTD | s2src f32 | s2dst f32]
                pst = epp2.tile([128, 128], BF16, tag="pst")
                nc.tensor.transpose(pst[:], elu[:], idt[:])
                eluT = bp.tile([128, 128], BF16, tag="eluT")
                nc.vector.tensor_copy(eluT[:], pst[:])
                ps2 = epp2.tile([128, OUTD + 2], F32, tag="ps2")
                nc.tensor.matmul(ps2[:], eluT[:], w2t[:], start=True, stop=True)
                r2 = bp.tile([128, REC2], BF16, tag="r2")
                nc.scalar.activation(r2[:, :OUTD], ps2[:, :OUTD], AF.Copy)
                r232 = r2[:].bitcast(F32)
                nc.vector.tensor_copy(r232[:, OUTD // 2:OUTD // 2 + 1], ps2[:, OUTD:OUTD + 1])
                sd2 = bp.tile([128, 1], BF16, tag="sd2")
                nc.scalar.activation(sd2[:], ps2[:, OUTD + 1:OUTD + 2], AF.Copy)
                nc.sync.dma_start(rec2l[b * 128:b * 128 + dvalid, :REC2], r2[:dvalid, :])
                nc.sync.dma_start(dbg_d[b * 128:b * 128 + dvalid, :REC2], r2[:dvalid, :])
                nc.sync.dma_start(SD2[b * 128:b * 128 + dvalid, :], sd2[:dvalid, :])

            edge_layer(1, T1, REC1, H1, F1, None, fin1)

            # ---------- allgather rec2 ----------
            nc.gpsimd.collective_compute(
                "AllGather", OP.bypass, replica_groups=[list(range(NCORES))],
                ins=[rec2l[:, :]], outs=[T2[:, :]])

            # ---------- L2 finalize: normalize + bias + log_softmax ----------
            def fin2(b, psb, dvalid):
                r = bp.tile([128, 1], F32, tag="rr2")
                nc.vector.reciprocal(r[:, :], psb[:, OUTD:OUTD + 1])
                o2 = bp.tile([128, OUTD], F32, tag="o2")
                nc.vector.tensor_scalar(o2[:], psb[:, :OUTD], r[:, :1], None, op0=OP.mult)
                nc.vector.tensor_tensor(o2[:], o2[:], b2t[:], op=OP.add)
                mx = bp.tile([128, 1], F32, tag="mx")
                nc.vector.tensor_reduce(mx[:], o2[:], axis=mybir.AxisListType.X, op=OP.max)
                nc.vector.tensor_scalar(o2[:], o2[:], mx[:, :1], None, op0=OP.subtract)
                exp2 = bp.tile([128, OUTD], F32, tag="exp2")
                nc.scalar.activation(exp2[:], o2[:], AF.Exp)
                sm = bp.tile([128, 1], F32, tag="sm")
                nc.vector.tensor_reduce(sm[:], exp2[:], axis=mybir.AxisListType.X, op=OP.add)
                nc.scalar.activation(sm[:], sm[:], AF.Ln)
                nc.vector.tensor_scalar(o2[:], o2[:], sm[:, :1], None, op0=OP.subtract)
                nc.sync.dma_start(out_d[b * 128:b * 128 + dvalid, :], o2[:dvalid, :])

            edge_layer(2, T2, REC2, 1, OUTD, SD2, fin2)

    nc.finalize()
    return nc


def kernel(**inputs):
    n = int(np.asarray(inputs["x"]).shape[0])
    consts, per_core, meta = build_host_data(inputs, n, NCORES)
    nc = build_kernel(meta)
    in_maps = []
    for c in range(NCORES):
        m = dict(consts)
        m.update(per_core[c])
        m = {k: np.ascontiguousarray(v) for k, v in m.items()}
        in_maps.append({
            "xT": m["xT"], "xTl": np.ascontiguousarray(m["xT"][:, c * meta["nloc"]:(c + 1) * meta["nloc"]]),
            "W1cat": m["W1cat"], "W2cat": m["W2cat"],
            "iota_row": m["iota_row"], "iota_col": m["iota_col"], "ident": m["ident"],
            "b1_bcast": m["b1_bcast"], "b2_bcast": m["b2_bcast"],
            "idxA": m["idxA"], "idxB": m["idxB"], "segcol": m["segcol"],
            "segbcast": m["segbcast"],
        })
    import os
    trace = bool(int(os.environ.get("GAT_TRACE", "0")))
    kw = dict(trace=True, tmpdir=os.environ.get("GAT_TRACEDIR", "/tmp/gat_trace")) if trace else {}
    res = bass_utils.run_bass_kernel_spmd(nc, in_maps, core_ids=list(range(NCORES)), **kw)
    global LAST_EXEC_NS
    LAST_EXEC_NS = res.exec_time_ns
    out = np.concatenate([res.results[c]["out"] for c in range(NCORES)], axis=0)
    return out.astype(np.float32)


if __name__ == "__main__":
    pass


# revision 15
# speedup vs baseline: 1.1680x; 1.1680x over previous
"""GATNet (2-layer GAT, 8 heads->1 head) on 8 trn2 NeuronCores.

Strategy: dst-node sharding (pull mode). Each core owns N/8 dst nodes and
all edges into them (sorted by dst, grouped into 128-dst blocks, 128-edge
chunks). Dense per-node work (x@W1, attention dots) is replicated on all
cores; per-edge work uses dma_gather (SWDGE) of packed per-node records,
segment-softmax via indicator matmuls accumulating in PSUM, and one
AllGather at the layer boundary for the layer-2 node records.
"""
import inspect
import textwrap

import ml_dtypes
import numpy as np

import concourse.bass as bass
import concourse.bacc as bacc
import concourse.mybir as mybir
import concourse.tile as tile
from concourse import bass_utils
from concourse.bass import AP

# ---- relax dma_gather's elem_size%256 assert (stride stays %256 = ISA field;
# 288B/36B payloads verified on HW) ----
_src = textwrap.dedent(inspect.getsource(bass.BassGpSimd.dma_gather))
_old = """    assert (
        elem_size_bytes > 0 and elem_size_bytes % 256 == 0
    )  # transpose restriction"""
_new = "    assert elem_size_bytes > 0 and (elem_size_bytes % 256 == 0 or (not transpose and elem_size_bytes % 4 == 0))"
assert _old in _src
_ns = {}
exec(compile(_src.replace(_old, _new), "<patched_dma_gather>", "exec"), bass.__dict__, _ns)
bass.BassGpSimd.dma_gather = _ns["dma_gather"]

F32 = mybir.dt.float32
BF16 = mybir.dt.bfloat16
I16 = mybir.dt.int16
I8 = mybir.dt.int8
AF = mybir.ActivationFunctionType
OP = mybir.AluOpType

NEG = 0.2
NCORES = 8
LAST_EXEC_NS = None
SHARD = 32768  # int16 index limit


def _ceil(a, b):
    return -(-a // b)


def _prep_core(src, dst, lo, hi, nblk):
    """Per-core host prep: edges with dst in [lo,hi), dst-sorted on entry.
    Returns per-block chunk lists: (idxA, idxB, segcol, nA, nB) unpadded."""
    blocks = []
    for b in range(nblk):
        dlo, dhi = lo + b * 128, min(lo + (b + 1) * 128, hi)
        i0, i1 = np.searchsorted(dst, [dlo, dhi])
        s, d = src[i0:i1], dst[i0:i1]
        a_mask = s < SHARD
        sa, da = s[a_mask], d[a_mask]
        sb, db = s[~a_mask] - SHARD, d[~a_mask]
        blocks.append(((sa, da - dlo + 0), (sb, db - dlo), dlo - lo))
    return blocks


def _pad_chunks(s, seg, nchunks):
    """Pad edge list to nchunks*128; idx=0, segid=-1 for pad lanes."""
    tot = nchunks * 128
    idx = np.zeros(tot, np.int16)
    sg = np.full(tot, -1.0, np.float32)
    idx[: len(s)] = s.astype(np.int16)
    sg[: len(s)] = seg.astype(np.float32)
    return idx, sg


def _wrap_idx(idx):
    """[n*128] int16 -> [128, n*8] wrapped (16-partition) + replicated 8x."""
    n = len(idx) // 128
    w = idx.reshape(n * 8, 16).T  # [16, n*8]: pos (p, s) = idx[s*16+p]
    return np.tile(w, (8, 1)).copy()  # [128, n*8]


def build_host_data(inputs, n, ncores):
    x = np.asarray(inputs["x"], np.float32)
    ei = np.asarray(inputs["edge_index"])
    W1 = np.asarray(inputs["W1"], np.float32)
    a_src1 = np.asarray(inputs["a_src1"], np.float32)
    a_dst1 = np.asarray(inputs["a_dst1"], np.float32)
    b1 = np.asarray(inputs["b1"], np.float32)
    W2 = np.asarray(inputs["W2"], np.float32)
    a_src2 = np.asarray(inputs["a_src2"], np.float32)
    a_dst2 = np.asarray(inputs["a_dst2"], np.float32)
    b2 = np.asarray(inputs["b2"], np.float32)
    H1, HID = a_src1.shape
    OUTD = a_src2.shape[1]

    loop = np.arange(n, dtype=np.int64)
    src = np.concatenate([ei[0].astype(np.int64), loop])
    dst = np.concatenate([ei[1].astype(np.int64), loop])
    order = np.argsort(dst, kind="stable")
    src, dst = src[order], dst[order]

    nloc = n // ncores
    nblk = _ceil(nloc, 128)
    cores = [_prep_core(src, dst, c * nloc, (c + 1) * nloc, nblk) for c in range(ncores)]

    # uniform (ncA, ncB) per block across cores (SPMD: one program)
    ncA = [max(_ceil(max(len(cores[c][b][0][0]), 1), 128) for c in range(ncores)) for b in range(nblk)]
    ncB = [max(_ceil(max(len(cores[c][b][1][0]), 1), 128) for c in range(ncores)) for b in range(nblk)]

    # flat streams per core
    per_core = []
    for c in range(ncores):
        idxA_l, idxB_l, segc_l, segb_l = [], [], [], []
        for b in range(nblk):
            (sa, ga), (sb, gb), _ = cores[c][b]
            ia, sga = _pad_chunks(sa, ga, ncA[b])
            ib, sgb = _pad_chunks(sb, gb, ncB[b])
            idxA_l.append(_wrap_idx(ia))
            idxB_l.append(_wrap_idx(ib))
            seg = np.concatenate([sga, sgb])  # [(ncA+ncB)*128]
            nc_t = ncA[b] + ncB[b]
            segc_l.append(seg.reshape(nc_t, 128).T.copy())  # [128 lanes, nc_t]
            segb_l.append(np.broadcast_to(seg.astype(ml_dtypes.bfloat16).reshape(1, nc_t * 128), (128, nc_t * 128)))
        per_core.append(dict(
            idxA=np.concatenate(idxA_l, axis=1),          # [128, sum(ncA)*8] i16
            idxB=np.concatenate(idxB_l, axis=1),          # [128, sum(ncB)*8] i16
            segcol=np.concatenate(segc_l, axis=1).astype(np.float32),  # [128, sum(nc)]
            segbcast=np.concatenate(segb_l, axis=1).astype(ml_dtypes.bfloat16),
        ))

    # fused weights
    # s_src1[n,h] = sum_c h[n,h,c]*a_src1[h,c] = x[n] @ (W1[:, h*HID:(h+1)*HID] @ a_src1[h])
    wsrc1 = np.stack([W1[:, h * HID:(h + 1) * HID] @ a_src1[h] for h in range(H1)], axis=1)  # [128, H1]
    wdst1 = np.stack([W1[:, h * HID:(h + 1) * HID] @ a_dst1[h] for h in range(H1)], axis=1)
    W1cat = np.concatenate([W1, wsrc1, wdst1], axis=1)  # [128, 144]
    W2cat = np.concatenate([W2, W2 @ a_src2[0][:, None], W2 @ a_dst2[0][:, None]], axis=1)  # [128, 18]

    consts = dict(
        xT=np.ascontiguousarray(x.T).astype(ml_dtypes.bfloat16),  # [128, n]
        W1cat=W1cat.astype(ml_dtypes.bfloat16),
        W2cat=W2cat.astype(ml_dtypes.bfloat16),
        iota_row=np.broadcast_to(np.arange(128, dtype=np.float32), (128, 128)).copy(),
        iota_col=np.arange(128, dtype=np.float32).reshape(128, 1).copy(),
        ident=np.eye(128, dtype=np.float32).astype(ml_dtypes.bfloat16),
        b1_bcast=np.broadcast_to(b1, (128, H1 * HID)).copy().astype(np.float32),
        b2_bcast=np.broadcast_to(b2, (128, OUTD)).copy().astype(np.float32),
    )
    meta = dict(n=n, nloc=nloc, nblk=nblk, ncA=ncA, ncB=ncB, H1=H1, HID=HID, OUTD=OUTD,
                totA=sum(ncA), totB=sum(ncB), tot=sum(ncA) + sum(ncB))
    return consts, per_core, meta


def bcast_inner(ap, count):
    """Append a step-0 dim of size count to an AP (free-dim broadcast)."""
    return AP(ap.tensor, ap.offset, list(ap.ap) + [[0, count]])


def build_kernel(meta):
    n, nloc, nblk = meta["n"], meta["nloc"], meta["nblk"]
    H1, HID, OUTD = meta["H1"], meta["HID"], meta["OUTD"]
    F1 = H1 * HID  # 128
    ncA, ncB = meta["ncA"], meta["ncB"]
    NCMAX = max(ncA[b] + ncB[b] for b in range(nblk))
    NTILE = _ceil(n, 128)

    nc = bacc.Bacc(None, target_bir_lowering=False, num_devices=NCORES, num_swdge_queues=4)
    # inputs
    xT = nc.dram_tensor("xT", [128, n], BF16, kind="ExternalInput")
    W1c = nc.dram_tensor("W1cat", [128, F1 + 2 * H1], BF16, kind="ExternalInput")
    W2c = nc.dram_tensor("W2cat", [F1, OUTD + 2], BF16, kind="ExternalInput")
    iota_row_d = nc.dram_tensor("iota_row", [128, 128], F32, kind="ExternalInput")
    iota_col_d = nc.dram_tensor("iota_col", [128, 1], F32, kind="ExternalInput")
    ident_d = nc.dram_tensor("ident", [128, 128], BF16, kind="ExternalInput")
    b1_d = nc.dram_tensor("b1_bcast", [128, F1], F32, kind="ExternalInput")
    b2_d = nc.dram_tensor("b2_bcast", [128, OUTD], F32, kind="ExternalInput")
    xTl = nc.dram_tensor("xTl", [128, nloc], BF16, kind="ExternalInput")
    idxA_d = nc.dram_tensor("idxA", [128, meta["totA"] * 8], I16, kind="ExternalInput")
    idxB_d = nc.dram_tensor("idxB", [128, meta["totB"] * 8], I16, kind="ExternalInput")
    segcol_d = nc.dram_tensor("segcol", [128, meta["tot"]], F32, kind="ExternalInput")
    segb_d = nc.dram_tensor("segbcast", [128, meta["tot"] * 128], BF16, kind="ExternalInput")
    out_d = nc.dram_tensor("out", [nloc, OUTD], F32, kind="ExternalOutput")
    dbg_d = nc.dram_tensor("dbg_rec2", [nloc, 24], BF16, kind="ExternalOutput")
    dbg2_d = nc.dram_tensor("dbg_den", [nloc, 8], F32, kind="ExternalOutput")
    # internal DRAM
    T1 = nc.dram_tensor("T1", [NTILE * 128, 256], BF16, kind="Internal")
    rec2l = nc.dram_tensor("rec2l", [nloc, 128], BF16, kind="Internal")
    SD2 = nc.dram_tensor("SD2", [nblk * 128, 1], BF16, kind="Internal")
    T2 = nc.dram_tensor("T2", [n, 128], BF16, kind="Internal", addr_space="Shared")

    REC1 = F1 + 2 * H1  # 144 bf16 slots (h bf16 + s_src f32x8 as 16 slots)
    REC2 = OUTD + 4     # 20 bf16 slots (h2 bf16 x16 + s2src f32 as 2)... see below
    qn = [0]

    def nxq():
        qn[0] = (qn[0] + 1) % 4
        return qn[0]

    with tile.TileContext(nc) as tc:
        with (
            tc.tile_pool(name="const", bufs=1) as cp,
            tc.tile_pool(name="dense", bufs=3) as dp,
            tc.tile_pool(name="dpsum", bufs=2, space="PSUM") as dpp,
            tc.tile_pool(name="edge", bufs=4) as ep,
            tc.tile_pool(name="epsum", bufs=2, space="PSUM") as epp,
            tc.tile_pool(name="epsum2", bufs=1, space="PSUM") as epp2,
            tc.tile_pool(name="blk", bufs=3) as bp,
        ):
            w1t = cp.tile([128, F1 + 2 * H1], BF16)
            nc.sync.dma_start(w1t[:], W1c[:, :])
            w2t = cp.tile([F1, OUTD + 2], BF16)
            nc.sync.dma_start(w2t[:], W2c[:, :])
            irow = cp.tile([128, 128], F32)
            nc.sync.dma_start(irow[:], iota_row_d[:, :])
            icol = cp.tile([128, 1], F32)
            nc.sync.dma_start(icol[:], iota_col_d[:, :])
            idt = cp.tile([128, 128], BF16)
            nc.sync.dma_start(idt[:], ident_d[:, :])
            b1t = cp.tile([128, F1], F32)
            nc.sync.dma_start(b1t[:], b1_d[:, :])
            b2t = cp.tile([128, OUTD], F32)
            nc.sync.dma_start(b2t[:], b2_d[:, :])

            # ---------- dense phase: T1 records + SD1, all nodes ----------
            for t in range(NTILE):
                m = min(128, n - t * 128)
                xt = dp.tile([128, 128], BF16, tag="xt")
                nc.sync.dma_start(xt[:, :m], xT[:, t * 128:t * 128 + m])
                ps = dpp.tile([128, F1 + 2 * H1], F32, tag="dps")
                nc.tensor.matmul(ps[:m, :], xt[:, :m], w1t[:], start=True, stop=True)
                stg = dp.tile([128, REC1], BF16, tag="stg")
                nc.scalar.activation(stg[:m, :F1], ps[:m, :F1], AF.Copy)
                stg32 = stg[:].bitcast(F32)
                nc.vector.tensor_copy(stg32[:m, F1 // 2:F1 // 2 + H1], ps[:m, F1:F1 + H1])
                nc.sync.dma_start(T1[t * 128:t * 128 + m, :REC1], stg[:m, :])

            # ---------- edge phase helper ----------
            def edge_layer(layer, T, rec_w, nheads, fdim, SD, finalize):
                """rec_w: bf16 slots gathered per edge; h part = fdim*? ..."""
                offA = offB = offC = 0
                for b in range(nblk):
                    na, nb_ = ncA[b], ncB[b]
                    nct = na + nb_
                    dvalid = min(128, nloc - b * 128)
                    # block loads
                    sdb = bp.tile([128, nheads], BF16, tag=f"sdb{layer}")
                    if dvalid < 128:
                        nc.vector.memset(sdb[:, :], 0.0)
                    if layer == 1:
                        xb = bp.tile([128, 128], BF16, tag="xb")
                        nc.sync.dma_start(xb[:, :dvalid], xTl[:, b * 128:b * 128 + dvalid])
                        psdb = epp2.tile([128, OUTD + 2], F32, tag="ps2")
                        nc.tensor.matmul(psdb[:dvalid, :nheads], xb[:, :dvalid],
                                         w1t[:, F1 + H1:F1 + 2 * H1], start=True, stop=True)
                        nc.scalar.activation(sdb[:dvalid, :], psdb[:dvalid, :nheads], AF.Copy)
                    else:
                        nc.sync.dma_start(sdb[:dvalid, :], SD[b * 128:b * 128 + dvalid, :])
                    it = ep.tile([128, NCMAX * 8], I16, tag=f"it{layer}")
                    nc.sync.dma_start(it[:, :na * 8], idxA_d[:, offA * 8:(offA + na) * 8])
                    nc.sync.dma_start(it[:, na * 8:nct * 8], idxB_d[:, offB * 8:(offB + nb_) * 8])
                    scol = ep.tile([128, NCMAX], F32, tag=f"sc{layer}")
                    nc.sync.dma_start(scol[:, :nct], segcol_d[:, offC:offC + nct])
                    sbc = ep.tile([128, NCMAX * 128], BF16, tag=f"sb{layer}")
                    nc.sync.dma_start(sbc[:, :nct * 128], segb_d[:, offC * 128:(offC + nct) * 128])
                    # gathers (A then B shard)
                    g = ep.tile([128, NCMAX * rec_w], BF16, tag=f"g{layer}")
                    TA = T[:SHARD, :rec_w] if T.shape[0] > SHARD else T[:, :rec_w]
                    nc.gpsimd.dma_gather(
                        g[:, :na * rec_w].rearrange("p (t e) -> p t e", e=rec_w),
                        TA, it[:, :na * 8],
                        num_idxs=na * 128, num_idxs_reg=na * 128, elem_size=rec_w,
                        elem_step=T.shape[1], single_packet=False, queue_num=nxq())
                    if n > SHARD:
                        nc.gpsimd.dma_gather(
                            g[:, na * rec_w:nct * rec_w].rearrange("p (t e) -> p t e", e=rec_w),
                            T[SHARD:, :rec_w], it[:, na * 8:nct * 8],
                            num_idxs=nb_ * 128, num_idxs_reg=nb_ * 128, elem_size=rec_w,
                            elem_step=T.shape[1], single_packet=False, queue_num=nxq())
                    else:
                        nc.gpsimd.dma_gather(
                            g[:, na * rec_w:nct * rec_w].rearrange("p (t e) -> p t e", e=rec_w),
                            T[:, :rec_w], it[:, na * 8:nct * 8],
                            num_idxs=nb_ * 128, num_idxs_reg=nb_ * 128, elem_size=rec_w,
                            elem_step=T.shape[1], single_packet=False, queue_num=nxq())
                    # indicators
                    indT = ep.tile([128, NCMAX * 128], BF16, tag=f"iT{layer}")
                    nc.vector.tensor_scalar(indT[:, :nct * 128], sbc[:, :nct * 128], icol[:, :1], None, op0=OP.is_equal)
                    ind = ep.tile([128, NCMAX * 128], BF16, tag=f"ind{layer}")
                    nc.vector.tensor_tensor(
                        ind[:].rearrange("p (c e) -> p c e", e=128)[:, :nct, :],
                        bcast_inner(scol[:, :nct], 128),
                        AP(irow[:].tensor, irow[:].offset, [irow[:].ap[0], [0, nct], irow[:].ap[1]]),
                        op=OP.is_equal)
                    # s_dst expansion: per chunk matmul
                    pssd = epp.tile([128, NCMAX * 8], F32, tag="psd")
                    for ci in range(nct):
                        nc.tensor.matmul(pssd[:, ci * nheads:(ci + 1) * nheads],
                                         indT[:, ci * 128:(ci + 1) * 128], sdb[:, :],
                                         start=True, stop=True)
                    sD = ep.tile([128, NCMAX * nheads], F32, tag=f"sD{layer}")
                    nc.vector.tensor_copy(sD[:, :nct * nheads], pssd[:, :nct * nheads])
                    # e = lrelu(sS + sD); ee = exp(e)  (batched per block)
                    g32 = g[:].bitcast(F32)
                    sSv = g32[:, :nct * (rec_w // 2)].rearrange(
                        "p (c w) -> p c w", w=rec_w // 2)[:, :, fdim // 2:fdim // 2 + nheads]
                    et = ep.tile([128, NCMAX * nheads], F32, tag=f"et{layer}")
                    nc.vector.tensor_tensor(
                        et[:, :nct * nheads].rearrange("p (c h) -> p c h", h=nheads),
                        sSv, sD[:, :nct * nheads].rearrange("p (c h) -> p c h", h=nheads), op=OP.add)
                    nc.vector.scalar_tensor_tensor(et[:, :nct * nheads], et[:, :nct * nheads], NEG,
                                                   et[:, :nct * nheads], op0=OP.mult, op1=OP.max)
                    vw = fdim + nheads
                    chan = fdim // nheads
                    EE = ep.tile([128, NCMAX * nheads], BF16, tag=f"EE{layer}")
                    nc.scalar.activation(EE[:, :nct * nheads], et[:, :nct * nheads], AF.Exp)
                    V = ep.tile([128, NCMAX * fdim], BF16, tag=f"V{layer}")
                    nc.vector.tensor_tensor(
                        V[:, :nct * fdim].rearrange("p (c h x) -> p c h x", h=nheads, x=chan),
                        g[:, :nct * rec_w].rearrange("p (c w) -> p c w", w=rec_w)[:, :, :fdim].rearrange("p c (h x) -> p c h x", x=chan),
                        bcast_inner(EE[:, :nct * nheads].rearrange("p (c h) -> p c h", h=nheads), chan),
                        op=OP.mult)
                    # aggregation
                    psb = epp.tile([128, 136], F32, tag="pb")
                    for ci in range(nct):
                        nc.tensor.matmul(psb[:, :fdim], ind[:, ci * 128:(ci + 1) * 128],
                                         V[:, ci * fdim:(ci + 1) * fdim],
                                         start=(ci == 0), stop=(ci == nct - 1))
                    for ci in range(nct):
                        nc.tensor.matmul(psb[:, fdim:vw], ind[:, ci * 128:(ci + 1) * 128],
                                         EE[:, ci * nheads:(ci + 1) * nheads],
                                         start=(ci == 0), stop=(ci == nct - 1))
                    finalize(b, psb, dvalid)
                    offA += na; offB += nb_; offC += nct

            # ---------- L1 finalize: normalize + elu -> rec2 ----------
            def fin1(b, psb, dvalid):
                dbt = bp.tile([128, H1], F32, tag="dbt")
                nc.vector.tensor_copy(dbt[:], psb[:, F1:F1 + H1])
                nc.sync.dma_start(dbg2_d[b * 128:b * 128 + dvalid, :], dbt[:dvalid, :])
                r = bp.tile([128, H1], F32, tag="r1")
                nc.vector.reciprocal(r[:, :], psb[:, F1:F1 + H1])
                o1 = bp.tile([128, F1], F32, tag="o1")
                nc.vector.tensor_tensor(
                    o1[:].rearrange("p (h x) -> p h x", x=HID),
                    psb[:, :F1].rearrange("p (h x) -> p h x", x=HID),
                    bcast_inner(r[:, :], HID), op=OP.mult)
                nc.vector.tensor_tensor(o1[:], o1[:], b1t[:], op=OP.add)
                # elu = max(x,0) + exp(min(x,0)) - 1
                neg = bp.tile([128, F1], F32, tag="neg")
                nc.vector.tensor_scalar(neg[:], o1[:], 0.0, None, op0=OP.min)
                ex = bp.tile([128, F1], F32, tag="ex")
                nc.scalar.activation(ex[:], neg[:], AF.Exp)
                nc.vector.tensor_scalar(o1[:], o1[:], 0.0, None, op0=OP.max)
                elu = bp.tile([128, F1], BF16, tag="elu")
                nc.vector.scalar_tensor_tensor(elu[:], o1[:], -1.0, ex[:], op0=OP.add, op1=OP.add)
                # transpose -> rec2 = [h2 bf16 x OUTD | s2src f32 | s2dst f32]
                pst = epp2.tile([128, 128], BF16, tag="pst")
                nc.tensor.transpose(pst[:], elu[:], idt[:])
                eluT = bp.tile([128, 128], BF16, tag="eluT")
                nc.vector.tensor_copy(eluT[:], pst[:])
                ps2 = epp2.tile([128, OUTD + 2], F32, tag="ps2")
                nc.tensor.matmul(ps2[:], eluT[:], w2t[:], start=True, stop=True)
                r2 = bp.tile([128, REC2], BF16, tag="r2")
                nc.scalar.activation(r2[:, :OUTD], ps2[:, :OUTD], AF.Copy)
                r232 = r2[:].bitcast(F32)
                nc.vector.tensor_copy(r232[:, OUTD // 2:OUTD // 2 + 1], ps2[:, OUTD:OUTD + 1])
                sd2 = bp.tile([128, 1], BF16, tag="sd2")
                nc.scalar.activation(sd2[:], ps2[:, OUTD + 1:OUTD + 2], AF.Copy)
                nc.sync.dma_start(rec2l[b * 128:b * 128 + dvalid, :REC2], r2[:dvalid, :])
                nc.sync.dma_start(dbg_d[b * 128:b * 128 + dvalid, :REC2], r2[:dvalid, :])
                nc.sync.dma_start(SD2[b * 128:b * 128 + dvalid, :], sd2[:dvalid, :])

            edge_layer(1, T1, REC1, H1, F1, None, fin1)

            # ---------- allgather rec2 ----------
            nc.gpsimd.collective_compute(
                "AllGather", OP.bypass, replica_groups=[list(range(NCORES))],
                ins=[rec2l[:, :]], outs=[T2[:, :]])

            # ---------- L2 finalize: normalize + bias + log_softmax ----------
            def fin2(b, psb, dvalid):
                r = bp.tile([128, 1], F32, tag="rr2")
                nc.vector.reciprocal(r[:, :], psb[:, OUTD:OUTD + 1])
                o2 = bp.tile([128, OUTD], F32, tag="o2")
                nc.vector.tensor_scalar(o2[:], psb[:, :OUTD], r[:, :1], None, op0=OP.mult)
                nc.vector.tensor_tensor(o2[:], o2[:], b2t[:], op=OP.add)
                mx = bp.tile([128, 1], F32, tag="mx")
                nc.vector.tensor_reduce(mx[:], o2[:], axis=mybir.AxisListType.X, op=OP.max)
                nc.vector.tensor_scalar(o2[:], o2[:], mx[:, :1], None, op0=OP.subtract)
                exp2 = bp.tile([128, OUTD], F32, tag="exp2")
                nc.scalar.activation(exp2[:], o2[:], AF.Exp)
                sm = bp.tile([128, 1], F32, tag="sm")
                nc.vector.tensor_reduce(sm[:], exp2[:], axis=mybir.AxisListType.X, op=OP.add)
                nc.scalar.activation(sm[:], sm[:], AF.Ln)
                nc.vector.tensor_scalar(o2[:], o2[:], sm[:, :1], None, op0=OP.subtract)
                nc.sync.dma_start(out_d[b * 128:b * 128 + dvalid, :], o2[:dvalid, :])

            edge_layer(2, T2, REC2, 1, OUTD, SD2, fin2)

    nc.finalize()
    return nc


def kernel(**inputs):
    n = int(np.asarray(inputs["x"]).shape[0])
    consts, per_core, meta = build_host_data(inputs, n, NCORES)
    nc = build_kernel(meta)
    in_maps = []
    for c in range(NCORES):
        m = dict(consts)
        m.update(per_core[c])
        m = {k: np.ascontiguousarray(v) for k, v in m.items()}
        in_maps.append({
            "xT": m["xT"], "xTl": np.ascontiguousarray(m["xT"][:, c * meta["nloc"]:(c + 1) * meta["nloc"]]),
            "W1cat": m["W1cat"], "W2cat": m["W2cat"],
            "iota_row": m["iota_row"], "iota_col": m["iota_col"], "ident": m["ident"],
            "b1_bcast": m["b1_bcast"], "b2_bcast": m["b2_bcast"],
            "idxA": m["idxA"], "idxB": m["idxB"], "segcol": m["segcol"],
            "segbcast": m["segbcast"],
        })
    import os
    trace = bool(int(os.environ.get("GAT_TRACE", "0")))
    kw = dict(trace=True, tmpdir=os.environ.get("GAT_TRACEDIR", "/tmp/gat_trace")) if trace else {}
    res = bass_utils.run_bass_kernel_spmd(nc, in_maps, core_ids=list(range(NCORES)), **kw)
    global LAST_EXEC_NS
    LAST_EXEC_NS = res.exec_time_ns
    out = np.concatenate([res.results[c]["out"] for c in range(NCORES)], axis=0)
    return out.astype(np.float32)


if __name__ == "__main__":
    pass
